# revision 1
# baseline (speedup 1.0000x reference)
"""Trainium2 Bass kernel for nn_MinkUNet — fused single-launch version.

One SPMD bass module on 8 NeuronCores runs the whole network:
  vox(segment-matmul) -> AG -> conv1 -> AR(bn) -> AG -> conv2 -> AR -> AG
      -> r1 -> AR -> AG -> r2(+res+cls) -> AR -> AG -> devox
Tables are exchanged with on-device AllGather; BN statistics with AllReduce.
Voxelize uses an on-device one-hot segment matrix (no gathers); convs and
devox use [128,1]-offset indirect DMA gathers (the only HW-supported form).
"""
import numpy as np

import concourse.bass as bass
import concourse.mybir as mybir
from concourse.tile import TileContext
from concourse.masks import make_identity

f32 = mybir.dt.float32
i32 = mybir.dt.int32
ACT = mybir.ActivationFunctionType
ALU = mybir.AluOpType

# problem sizes (hardcoded per contract)
N, M, K, KD = 400000, 300000, 27, 8
CIN, C0, NCLS = 4, 32, 19
EPS = 1e-5
NC = 8
Ms = M // NC                      # 37500
MsP = 296 * 128                   # 37888 = 74*512
MT = NC * MsP                     # 303104
Np = N // NC                      # 50000
NpP = 392 * 128                   # 50176 = 98*512
ZR = Ms                           # zero row (shard-0 pad row 0) in padded coords
SUP = 4                           # tiles per supertile
NSUP_V = MsP // (SUP * 128)       # 74
NSUP_P = NpP // (SUP * 128)       # 98
PSEG = 1024                       # padded points per voxel-supertile
USEG = PSEG // 128                # 8
RG = [list(range(NC))]

_cache = {}
LAUNCH_TIMES = []


# ---------------------------------------------------------------- wait splitting
def _split_sync_waits(bir_bytes, wait_limit=1):
    """Pinned walrus encodes at most 1 sync wait per instruction; split extras
    onto same-engine reg-move nops placed immediately before."""
    import json
    m = json.loads(bir_bytes)
    ctr = [0]

    def nop(engine, on_wait):
        ctr[0] += 1
        return {
            "debug": 0, "engine": engine,
            "ins": [{"dtype": "int32", "kind": "imm_value", "value": 0}],
            "outs": [{"dtype": "int32", "kind": "register_access",
                      "regref": f"{engine}_zero"}],
            "name": f"wsplit-{ctr[0]}", "opcode": "RegisterMove",
            "sync_info": {"on_wait": on_wait, "on_update": []},
        }

    for f in m["functions"]:
        for b in f["blocks"]:
            out = []
            for ins in b["instructions"]:
                si = ins.get("sync_info")
                if si:
                    ow = si.get("on_wait") or []
                    if len(ow) > wait_limit:
                        extra, keep = ow[:-wait_limit], ow[-wait_limit:]
                        for i in range(0, len(extra), wait_limit):
                            out.append(nop(ins["engine"], extra[i:i + wait_limit]))
                        si["on_wait"] = keep
                out.append(ins)
            b["instructions"] = out
    return json.dumps(m).encode()


def _install_waitfix(nc):
    orig = nc.to_json_bytes
    nc.to_json_bytes = lambda: _split_sync_waits(orig())
    return nc


# ---------------------------------------------------------------- SPMD runner
class _Runner:
    """jit once; inputs device_put per call; mirrors bass2jax multi-core path."""

    def __init__(self, nc):
        import jax
        from jax.sharding import Mesh, PartitionSpec, NamedSharding
        from jax.experimental.shard_map import shard_map
        from concourse import bass2jax
        from concourse.bass2jax import _bass_exec_p, install_neuronx_cc_hook
        install_neuronx_cc_hook()
        self.jax = jax
        self.nc = nc
        pname = nc.partition_id_tensor.name if nc.partition_id_tensor else None
        in_names, out_names, out_avals, zero_shapes = [], [], [], []
        for alloc in nc.m.functions[0].allocations:
            if not isinstance(alloc, mybir.MemoryLocationSet):
                continue
            name = alloc.memorylocations[0].name
            if alloc.kind == "ExternalInput":
                if name != pname:
                    in_names.append(name)
            elif alloc.kind == "ExternalOutput":
                out_names.append(name)
                shape = tuple(alloc.tensor_shape)
                dtype = mybir.dt.np(alloc.dtype)
                out_avals.append(jax.core.ShapedArray(shape, dtype))
                zero_shapes.append((shape, dtype))
        self.in_names, self.out_names, self.out_avals = in_names, out_names, out_avals
        all_in = list(in_names) + list(out_names)
        if pname is not None:
            all_in.append(pname)
        n_params, n_outs = len(in_names), len(out_names)

        def _body(*args):
            operands = list(args)
            if pname is not None:
                operands.append(bass2jax.partition_id_tensor())
            return tuple(_bass_exec_p.bind(
                *operands, out_avals=tuple(out_avals), in_names=tuple(all_in),
                out_names=tuple(out_names), lowering_input_output_aliases=(),
                sim_require_finite=True, sim_require_nnan=True, nc=nc))

        devices = jax.devices()[:NC]
        self.mesh = Mesh(np.asarray(devices), ("core",))
        specs_in = (PartitionSpec("core"),) * (n_params + n_outs)
        specs_out = (PartitionSpec("core"),) * n_outs
        self.fn = jax.jit(
            shard_map(_body, mesh=self.mesh, in_specs=specs_in,
                      out_specs=specs_out, check_rep=False),
            keep_unused=True)
        self.sharding = NamedSharding(self.mesh, PartitionSpec("core"))
        self.zeros = [
            self.jax.device_put(
                np.zeros((NC * s[0], *s[1:]), d), self.sharding)
            for s, d in zero_shapes
        ]

    def __call__(self, in_maps):
        concat = [
            np.concatenate([np.asarray(in_maps[c][n]) for c in range(NC)], 0)
            for n in self.in_names
        ]
        args = [self.jax.device_put(a, self.sharding) for a in concat]
        self.jax.block_until_ready(args)
        import time as _time
        _t0 = _time.perf_counter()
        outs = self.fn(*args, *self.zeros)
        self.jax.block_until_ready(outs)
        LAUNCH_TIMES.append(_time.perf_counter() - _t0)
        res = []
        for c in range(NC):
            res.append({
                n: np.asarray(outs[i]).reshape(NC, *self.out_avals[i].shape)[c]
                for i, n in enumerate(self.out_names)
            })
        return res


# ---------------------------------------------------------------- module build
_gq = [0]


def _gather(nc, out_ap, table_ap, idx_col):
    inst = nc.gpsimd.indirect_dma_start(
        out=out_ap, out_offset=None, in_=table_ap,
        in_offset=bass.IndirectOffsetOnAxis(ap=idx_col, axis=0))
    q = _gq[0] % 4
    _gq[0] += 1
    if q:
        inst.ins.queue = f"qPoolDynamic{q}"


def _bn_affine(nc, pool, st, g_sb, b_sb, tag):
    """st [32,2] (global sum, sumsq over M) -> (a, bb) [32,1] tiles."""
    mean = pool.tile([32, 1], f32, name=f"bnm{tag}")
    ex2 = pool.tile([32, 1], f32, name=f"bne{tag}")
    nc.vector.tensor_scalar_mul(mean[:], st[:, 0:1], 1.0 / M)
    nc.vector.tensor_scalar_mul(ex2[:], st[:, 1:2], 1.0 / M)
    m2 = pool.tile([32, 1], f32, name=f"bn2{tag}")
    nc.vector.tensor_tensor(out=m2[:], in0=mean[:], in1=mean[:], op=ALU.mult)
    var = pool.tile([32, 1], f32, name=f"bnv{tag}")
    nc.vector.tensor_tensor(out=var[:], in0=ex2[:], in1=m2[:], op=ALU.subtract)
    vp = pool.tile([32, 1], f32, name=f"bnp{tag}")
    nc.vector.tensor_scalar_add(vp[:], var[:], EPS)
    std = pool.tile([32, 1], f32, name=f"bns{tag}")
    nc.scalar.activation(out=std[:], in_=vp[:], func=ACT.Sqrt)
    inv = pool.tile([32, 1], f32, name=f"bni{tag}")
    nc.vector.reciprocal(inv[:], std[:])
    a = pool.tile([32, 1], f32, name=f"bna{tag}")
    nc.vector.tensor_tensor(out=a[:], in0=g_sb[:], in1=inv[:], op=ALU.mult)
    ma = pool.tile([32, 1], f32, name=f"bnq{tag}")
    nc.vector.tensor_tensor(out=ma[:], in0=mean[:], in1=a[:], op=ALU.mult)
    bb = pool.tile([32, 1], f32, name=f"bnb{tag}")
    nc.vector.tensor_tensor(out=bb[:], in0=b_sb[:], in1=ma[:], op=ALU.subtract)
    return a, bb


def build_fused(debug=False):
    nc = bass.Bass(num_swdge_queues=4)

    # ---- parameters
    pfseg = nc.declare_dram_parameter("pfseg", [NSUP_V * PSEG, CIN], f32,
                                      isOutput=False)
    pslot = nc.declare_dram_parameter("pslot", [NSUP_V * PSEG, 1], f32,
                                      isOutput=False)
    rcp = nc.declare_dram_parameter("rcp", [MsP, 1], f32, isOutput=False)
    nbrs = nc.declare_dram_parameter("nbrs", [MsP, 27], i32, isOutput=False)
    didx = nc.declare_dram_parameter("didx", [NpP, KD], i32, isOutput=False)
    wdev = nc.declare_dram_parameter("wdev", [NpP, KD], f32, isOutput=False)
    w1 = nc.declare_dram_parameter("w1", [128, C0], f32, isOutput=False)
    w2 = nc.declare_dram_parameter("w2", [7 * 128, C0], f32, isOutput=False)
    wr1 = nc.declare_dram_parameter("wr1", [7 * 128, C0], f32, isOutput=False)
    wr2 = nc.declare_dram_parameter("wr2", [7 * 128, C0], f32, isOutput=False)
    gb = nc.declare_dram_parameter("gb", [C0, 4 * 2], f32, isOutput=False)
    wc = nc.declare_dram_parameter("wc", [C0, C0], f32, isOutput=False)
    bc = nc.declare_dram_parameter("bc", [1, C0], f32, isOutput=False)
    iot5 = nc.declare_dram_parameter("iot5", [1, 512], f32, isOutput=False)
    pmask = nc.declare_dram_parameter("pmask", [128, SUP], f32, isOutput=False)
    out = nc.declare_dram_parameter("out", [NpP, NCLS], f32, isOutput=True)

    # ---- dram intermediates
    voxloc = nc.dram_tensor("voxloc", [MsP, CIN], f32)
    hloc = [nc.dram_tensor(f"hloc{i}", [MsP, C0], f32) for i in range(4)]
    rawT = [nc.dram_tensor(f"rawT{i}", [32, MsP], f32) for i in range(4)]
    voxfull = nc.dram_tensor("voxfull", [MT, CIN], f32, addr_space="Shared")
    hfull = [nc.dram_tensor(f"hfull{i}", [MT, C0], f32, addr_space="Shared")
             for i in range(4)]
    h2T = nc.dram_tensor("h2T", [32, MsP], f32)
    if debug:
        dvox = nc.declare_dram_parameter("dvox", [MsP, CIN], f32, isOutput=True)
        draw = [nc.declare_dram_parameter(f"draw{i}", [32, MsP], f32,
                                          isOutput=True) for i in range(4)]
        dh = [nc.declare_dram_parameter(f"dh{i}", [MsP, C0], f32,
                                        isOutput=True) for i in range(4)]
    stin = [nc.dram_tensor(f"stin{i}", [32, 2], f32) for i in range(4)]
    stout = [nc.dram_tensor(f"stout{i}", [32, 2], f32, addr_space="Shared")
             for i in range(4)]

    with TileContext(nc) as tc:
        with tc.tile_pool(name="const", bufs=1) as cp:
            ident = cp.tile([128, 128], f32, name="ident")
            make_identity(nc, ident[:])
            maskc = cp.tile([128, SUP], f32, name="maskc")
            nc.sync.dma_start(out=maskc[:], in_=pmask[:])
            gbsb = cp.tile([C0, 8], f32, name="gbsb")
            nc.sync.dma_start(out=gbsb[:], in_=gb[:])
            ones1 = cp.tile([1, 128], f32, name="ones1")
            nc.vector.memset(ones1[:], 1.0)

            # ---------------- stage 0: voxelize via segment matmul ----------
            with (
                tc.tile_pool(name="sbV", bufs=3) as sb,
                tc.tile_pool(name="ppV", bufs=2, space="PSUM") as pp,
            ):
                io5 = cp.tile([1, 512], f32, name="io5")
                nc.sync.dma_start(out=io5[:], in_=iot5[:])
                pio = pp.tile([128, 512], f32, name="pio")
                nc.tensor.matmul(out=pio[:], lhsT=ones1[:], rhs=io5[:],
                                 start=True, stop=True)
                iot = cp.tile([128, 512], f32, name="iot")
                nc.vector.tensor_copy(out=iot[:], in_=pio[:])

                pfs_r = pfseg[:].rearrange("(s u p) c -> s p u c", u=USEG, p=128)
                slt_r = pslot[:].rearrange("(s u p) o -> s p u o", u=USEG, p=128)
                rcp_r = rcp[:].rearrange("(s t p) o -> s p t o", t=SUP, p=128)
                vout_r = voxloc[:].rearrange("(s t p) c -> s p t c", t=SUP, p=128)
                for s in range(NSUP_V):
                    pfs = sb.tile([128, USEG * CIN], f32, name="pfs", tag="pfs")
                    nc.sync.dma_start(
                        out=pfs[:].rearrange("p (u c) -> p u c", u=USEG),
                        in_=pfs_r[s])
                    slt = sb.tile([128, USEG], f32, name="slt", tag="slt")
                    nc.sync.dma_start(
                        out=slt[:].rearrange("p (u o) -> p u o", u=USEG),
                        in_=slt_r[s])
                    pv = pp.tile([128, SUP * CIN], f32, name="pv", tag="pv")
                    # HW-probe finding: PSUM accumulation chains must be
                    # contiguous per output region — interleaving the four
                    # t-slices' start/stop groups (u-major order) silently
                    # corrupts results on silicon.  Build all USEG seg tiles
                    # first, then run each t-slice's 8-step chain to
                    # completion (t-major order).
                    segs = []
                    for u in range(USEG):
                        seg = sb.tile([128, 512], f32, name=f"seg{u}",
                                      tag=f"seg{u}")
                        scol = slt[:, u:u + 1]
                        so = bass.AP(scol.tensor, scol.offset,
                                     [list(scol.ap[0]), [0, 512]])
                        nc.vector.tensor_tensor(out=seg[:], in0=iot[:], in1=so,
                                                op=ALU.is_equal)
                        segs.append(seg)
                    for t in range(SUP):
                        for u in range(USEG):
                            nc.tensor.matmul(
                                out=pv[:, t * CIN:(t + 1) * CIN],
                                lhsT=segs[u][:, t * 128:(t + 1) * 128],
                                rhs=pfs[:, u * CIN:(u + 1) * CIN],
                                start=(u == 0), stop=(u == USEG - 1))
                    rc = sb.tile([128, SUP], f32, name="rcV", tag="rcV")
                    nc.sync.dma_start(
                        out=rc[:].rearrange("p (t o) -> p t o", t=SUP),
                        in_=rcp_r[s])
                    vsb = sb.tile([128, SUP * CIN], f32, name="vsbV", tag="vsbV")
                    rcb = bass.AP(rc[:].tensor, rc[:].offset,
                                  [list(rc[:].ap[0]), [1, SUP], [0, CIN]])
                    nc.vector.tensor_tensor(
                        out=vsb[:].rearrange("p (t c) -> p t c", t=SUP),
                        in0=pv[:].rearrange("p (t c) -> p t c", t=SUP),
                        in1=rcb, op=ALU.mult)
                    nc.sync.dma_start(
                        out=vout_r[s],
                        in_=vsb[:].rearrange("p (t c) -> p t c", t=SUP))
                    if debug:
                        dvox_r = dvox[:].rearrange("(s2 t p) c -> s2 p t c",
                                                   t=SUP, p=128)
                        nc.sync.dma_start(
                            out=dvox_r[s],
                            in_=vsb[:].rearrange("p (t c) -> p t c", t=SUP))
            nc.gpsimd.collective_compute(
                "AllGather", ALU.bypass, RG, ins=[voxloc[:]], outs=[voxfull[:]])

            # ---------------- conv stages ----------------
            def conv_stage(ci, table, wpar, nchunk, cin_cols, residual,
                           save_h2T, outloc, outfull):
                KK = 27
                GW = KK * cin_cols
                with (
                    tc.tile_pool(name=f"sbA{ci}", bufs=3) as sb,
                    tc.tile_pool(name=f"ppA{ci}", bufs=2, space="PSUM") as pp,
                ):
                    wsb = cp.tile([128, nchunk * C0], f32, name=f"wsb{ci}")
                    nc.sync.dma_start(
                        out=wsb[:].rearrange("p (j c) -> p j c", j=nchunk),
                        in_=wpar[:].rearrange("(j p) c -> p j c", p=128))
                    sums = cp.tile([32, NSUP_V], f32, name=f"sums{ci}")
                    sqs = cp.tile([32, NSUP_V], f32, name=f"sqs{ci}")
                    nbrs_r = nbrs[:].rearrange("(s t p) k -> s p t k", t=SUP, p=128)
                    for s in range(NSUP_V):
                        idx = sb.tile([128, SUP * KK], i32, name="idxA", tag="idxA")
                        nc.sync.dma_start(
                            out=idx[:].rearrange("p (t k) -> p t k", t=SUP),
                            in_=nbrs_r[s])
                        G = sb.tile([128, SUP * GW], f32, name="GA", tag="GA")
                        for t in range(SUP):
                            for k in range(KK):
                                _gather(
                                    nc,
                                    G[:, t * GW + k * cin_cols:
                                      t * GW + (k + 1) * cin_cols],
                                    table[:],
                                    idx[:, t * KK + k: t * KK + k + 1])
                        po = pp.tile([32, 512], f32, name="poA", tag="poA")
                        for j in range(nchunk):
                            pgt = pp.tile([128, 512], f32, name="pgtA", tag="pgtA")
                            cw = min(128, GW - j * 128)
                            if cw < 128:
                                nc.vector.memset(pgt[:], 0.0)
                            for t in range(SUP):
                                nc.tensor.transpose(
                                    out=pgt[:cw, t * 128:(t + 1) * 128],
                                    in_=G[:, t * GW + j * 128:
                                          t * GW + j * 128 + cw],
                                    identity=ident[:])
                            GT = sb.tile([128, 512], f32, name="GTA", tag="GTA")
                            nc.vector.tensor_copy(out=GT[:], in_=pgt[:])
                            nc.tensor.matmul(out=po[:],
                                             lhsT=wsb[:, j * C0:(j + 1) * C0],
                                             rhs=GT[:],
                                             start=(j == 0),
                                             stop=(j == nchunk - 1))
                        rawsb = sb.tile([32, 512], f32, name="rawA", tag="rawA")
                        nc.scalar.activation(out=rawsb[:], in_=po[:],
                                             func=ACT.Copy,
                                             accum_out=sums[:, s:s + 1])
                        sqsb = sb.tile([32, 512], f32, name="sqA", tag="sqA")
                        nc.vector.tensor_tensor(out=sqsb[:], in0=rawsb[:],
                                                in1=rawsb[:], op=ALU.mult)
                        nc.vector.tensor_reduce(out=sqs[:, s:s + 1], in_=sqsb[:],
                                                axis=mybir.AxisListType.X,
                                                op=ALU.add)
                        nc.sync.dma_start(out=rawT[ci][:, s * 512:(s + 1) * 512],
                                          in_=rawsb[:])
                        if debug:
                            nc.sync.dma_start(
                                out=draw[ci][:, s * 512:(s + 1) * 512],
                                in_=rawsb[:])
                    stats = cp.tile([32, 2], f32, name=f"st{ci}")
                    nc.vector.tensor_reduce(out=stats[:, 0:1], in_=sums[:],
                                            axis=mybir.AxisListType.X, op=ALU.add)
                    nc.vector.tensor_reduce(out=stats[:, 1:2], in_=sqs[:],
                                            axis=mybir.AxisListType.X, op=ALU.add)
                    nc.sync.dma_start(out=stin[ci][:], in_=stats[:])
                nc.gpsimd.collective_compute("AllReduce", ALU.add, RG,
                                             ins=[stin[ci][:]],
                                             outs=[stout[ci][:]])
                # pass B
                with (
                    tc.tile_pool(name=f"sbB{ci}", bufs=3) as sb,
                    tc.tile_pool(name=f"ppB{ci}", bufs=2, space="PSUM") as pp,
                ):
                    star = cp.tile([32, 2], f32, name=f"star{ci}")
                    nc.sync.dma_start(out=star[:], in_=stout[ci][:])
                    a, bb = _bn_affine(nc, cp, star,
                                       gbsb[:, 2 * ci:2 * ci + 1],
                                       gbsb[:, 2 * ci + 1:2 * ci + 2], ci)
                    if residual:
                        wcsb = cp.tile([C0, C0], f32, name="wcsb")
                        nc.sync.dma_start(out=wcsb[:], in_=wc[:])
                    hout_r = outloc[:].rearrange("(s t p) c -> s p t c",
                                                 t=SUP, p=128)
                    for s in range(NSUP_V):
                        raw2 = sb.tile([32, 512], f32, name="raw2", tag="raw2")
                        nc.sync.dma_start(out=raw2[:],
                                          in_=rawT[ci][:, s * 512:(s + 1) * 512])
                        if not residual:
                            hT = sb.tile([32, 512], f32, name="hT", tag="hT")
                            nc.scalar.activation(out=hT[:], in_=raw2[:],
                                                 func=ACT.Relu, bias=bb[:],
                                                 scale=a[:])
                            if save_h2T:
                                nc.sync.dma_start(
                                    out=h2T[:, s * 512:(s + 1) * 512], in_=hT[:])
                            ph = pp.tile([128, 128], f32, name="ph", tag="ph")
                            for t in range(SUP):
                                nc.tensor.transpose(
                                    out=ph[:, t * C0:(t + 1) * C0],
                                    in_=hT[:, t * 128:(t + 1) * 128],
                                    identity=ident[:32, :32])
                            hsb = sb.tile([128, 128], f32, name="hsb", tag="hsb")
                            if s == NSUP_V - 1:
                                mb = bass.AP(maskc[:].tensor, maskc[:].offset,
                                             [list(maskc[:].ap[0]),
                                              list(maskc[:].ap[1]), [0, C0]])
                                nc.vector.tensor_tensor(
                                    out=hsb[:].rearrange("p (t c) -> p t c",
                                                         t=SUP),
                                    in0=ph[:].rearrange("p (t c) -> p t c",
                                                        t=SUP),
                                    in1=mb, op=ALU.mult)
                            else:
                                nc.vector.tensor_copy(out=hsb[:], in_=ph[:])
                            nc.sync.dma_start(
                                out=hout_r[s],
                                in_=hsb[:].rearrange("p (t c) -> p t c", t=SUP))
                            if debug:
                                dh_r = dh[ci][:].rearrange(
                                    "(s2 t p) c -> s2 p t c", t=SUP, p=128)
                                nc.sync.dma_start(
                                    out=dh_r[s],
                                    in_=hsb[:].rearrange("p (t c) -> p t c",
                                                         t=SUP))
                        else:
                            t0 = sb.tile([32, 512], f32, name="t0", tag="t0")
                            nc.scalar.activation(out=t0[:], in_=raw2[:],
                                                 func=ACT.Identity, bias=bb[:],
                                                 scale=a[:])
                            h2sb = sb.tile([32, 512], f32, name="h2sb",
                                           tag="h2sb")
                            nc.sync.dma_start(
                                out=h2sb[:],
                                in_=h2T[:, s * 512:(s + 1) * 512])
                            s1 = sb.tile([32, 512], f32, name="s1", tag="s1")
                            nc.vector.tensor_tensor(out=s1[:], in0=t0[:],
                                                    in1=h2sb[:], op=ALU.add)
                            h3 = sb.tile([32, 512], f32, name="h3", tag="h3")
                            nc.vector.tensor_scalar_max(h3[:], s1[:], 0.0)
                            py = pp.tile([128, 128], f32, name="py", tag="py")
                            for t in range(SUP):
                                nc.tensor.matmul(
                                    out=py[:, t * C0:(t + 1) * C0],
                                    lhsT=h3[:, t * 128:(t + 1) * 128],
                                    rhs=wcsb[:], start=True, stop=True)
                            ysb = sb.tile([128, 128], f32, name="ysb", tag="ysb")
                            if s == NSUP_V - 1:
                                mb = bass.AP(maskc[:].tensor, maskc[:].offset,
                                             [list(maskc[:].ap[0]),
                                              list(maskc[:].ap[1]), [0, C0]])
                                nc.vector.tensor_tensor(
                                    out=ysb[:].rearrange("p (t c) -> p t c",
                                                         t=SUP),
                                    in0=py[:].rearrange("p (t c) -> p t c",
                                                        t=SUP),
                                    in1=mb, op=ALU.mult)
                            else:
                                nc.vector.tensor_copy(out=ysb[:], in_=py[:])
                            nc.sync.dma_start(
                                out=hout_r[s],
                                in_=ysb[:].rearrange("p (t c) -> p t c", t=SUP))
                            if debug:
                                dh_r = dh[ci][:].rearrange(
                                    "(s2 t p) c -> s2 p t c", t=SUP, p=128)
                                nc.sync.dma_start(
                                    out=dh_r[s],
                                    in_=ysb[:].rearrange("p (t c) -> p t c",
                                                         t=SUP))
                nc.gpsimd.collective_compute(
                    "AllGather", ALU.bypass, RG, ins=[outloc[:]],
                    outs=[outfull[:]])

            conv_stage(0, voxfull, w1, 1, CIN, False, False, hloc[0], hfull[0])
            conv_stage(1, hfull[0], w2, 7, C0, False, True, hloc[1], hfull[1])
            conv_stage(2, hfull[1], wr1, 7, C0, False, False, hloc[2], hfull[2])
            conv_stage(3, hfull[2], wr2, 7, C0, True, False, hloc[3], hfull[3])

            # ---------------- devox + classifier bias ----------------
            with (
                tc.tile_pool(name="sbD", bufs=3) as sb,
                tc.tile_pool(name="ppD", bufs=2, space="PSUM") as pp,
            ):
                bcs = cp.tile([1, C0], f32, name="bcs")
                nc.sync.dma_start(out=bcs[:], in_=bc[:])
                pbc = pp.tile([128, C0], f32, name="pbc")
                nc.tensor.matmul(out=pbc[:], lhsT=ones1[:], rhs=bcs[:],
                                 start=True, stop=True)
                bcb = cp.tile([128, C0], f32, name="bcb")
                nc.vector.tensor_copy(out=bcb[:], in_=pbc[:])

                didx_r = didx[:].rearrange("(s t p) k -> s p t k", t=SUP, p=128)
                wdev_r = wdev[:].rearrange("(s t p) k -> s p t k", t=SUP, p=128)
                out_r = out[:].rearrange("(s t p) c -> s p t c", t=SUP, p=128)
                GW = KD * C0
                ytab = hfull[3]
                for s in range(NSUP_P):
                    idx = sb.tile([128, SUP * KD], i32, name="idxD", tag="idxD")
                    nc.sync.dma_start(
                        out=idx[:].rearrange("p (t k) -> p t k", t=SUP),
                        in_=didx_r[s])
                    G = sb.tile([128, SUP * GW], f32, name="GD", tag="GD")
                    for t in range(SUP):
                        for k in range(KD):
                            _gather(nc,
                                    G[:, t * GW + k * C0: t * GW + (k + 1) * C0],
                                    ytab[:], idx[:, t * KD + k: t * KD + k + 1])
                    w4 = sb.tile([128, SUP * KD], f32, name="w4", tag="w4")
                    nc.sync.dma_start(
                        out=w4[:].rearrange("p (t k) -> p t k", t=SUP),
                        in_=wdev_r[s])
                    prod = sb.tile([128, SUP * GW], f32, name="prod", tag="prod")
                    gv = G[:].rearrange("p (t k c) -> p t k c", t=SUP, k=KD, c=C0)
                    pvw = prod[:].rearrange("p (t c k) -> p t k c",
                                            t=SUP, c=C0, k=KD)
                    wv = w4[:].rearrange("p (t k) -> p t k", t=SUP)
                    wb = bass.AP(wv.tensor, wv.offset,
                                 [list(wv.ap[0]), list(wv.ap[1]),
                                  list(wv.ap[2]), [0, C0]])
                    nc.vector.tensor_tensor(out=pvw, in0=gv, in1=wb, op=ALU.mult)
                    pts = sb.tile([128, SUP * C0], f32, name="pts", tag="pts")
                    nc.vector.tensor_reduce(
                        out=pts[:].rearrange("p (t c) -> p t c", t=SUP),
                        in_=prod[:].rearrange("p (t c k) -> p t c k",
                                              t=SUP, c=C0, k=KD),
                        axis=mybir.AxisListType.X, op=ALU.add)
                    res = sb.tile([128, SUP * C0], f32, name="res", tag="res")
                    bcv = bass.AP(bcb[:].tensor, bcb[:].offset,
                                  [list(bcb[:].ap[0]), [0, SUP],
                                   list(bcb[:].ap[1])])
                    nc.vector.tensor_tensor(
                        out=res[:].rearrange("p (t c) -> p t c", t=SUP),
                        in0=pts[:].rearrange("p (t c) -> p t c", t=SUP),
                        in1=bcv, op=ALU.add)
                    nc.sync.dma_start(
                        out=out_r[s],
                        in_=res[:].rearrange("p (t c) -> p t c",
                                             t=SUP)[:, :, :NCLS])
    return _install_waitfix(nc)


# ---------------------------------------------------------------- host side
def _remap(g):
    g = np.asarray(g)
    gc = np.clip(g, 0, M - 1)
    s = gc // Ms
    out = s * MsP + (gc - s * Ms)
    return np.where(g < 0, ZR, out).astype(np.int32)


def _stack_w(Wk, nchunk):
    Wk = np.asarray(Wk, np.float32)
    kcin = Wk.shape[0] * Wk.shape[1]
    o = np.zeros((nchunk * 128, C0), np.float32)
    o[:kcin] = Wk.reshape(kcin, C0)
    return o


def _get_runner():
    if "fused" not in _cache:
        _cache["fused"] = _Runner(build_fused())
    return _cache["fused"]


def kernel(point_fea, idx_query, nbrs, idx_dev, w_dev,
           W_s1, W_s2, g_s1, b_s1, g_s2, b_s2,
           W_r1, W_r2, g_r1, b_r1, g_r2, b_r2, W_c, b_c):
    point_fea = np.asarray(point_fea, np.float32)
    idx_query = np.asarray(idx_query, np.int32)
    nbrs = np.asarray(nbrs, np.int32)
    idx_dev = np.asarray(idx_dev, np.int32)
    w_dev = np.asarray(w_dev, np.float32)

    # ---- host preprocessing (index plumbing only)
    counts = np.bincount(idx_query, minlength=M)
    order = np.argsort(idx_query, kind="stable")
    sorted_vox = idx_query[order]               # voxel id per sorted point
    starts = np.zeros(M + 1, np.int64)
    np.cumsum(counts, out=starts[1:])
    recip_full = (1.0 / np.maximum(counts, 1)).astype(np.float32)

    nb_remap = _remap(nbrs)                     # [M, 27]
    per = []
    for c in range(NC):
        vs = slice(c * Ms, (c + 1) * Ms)
        ps = slice(c * Np, (c + 1) * Np)
        # segment-packed points for this core's voxels
        pfseg = np.zeros((NSUP_V * PSEG, CIN), np.float32)
        pslot = np.full((NSUP_V * PSEG, 1), -1.0, np.float32)
        p0 = starts[c * Ms]
        for s in range(NSUP_V):
            v0 = c * Ms + s * 512
            v1 = min(v0 + 512, (c + 1) * Ms)
            if v0 >= (c + 1) * Ms:
                continue
            a, b = starts[v0], starts[v1]
            n = b - a
            assert n <= PSEG, f"supertile {s} of core {c} has {n} points"
            rows = order[a:b]
            pfseg[s * PSEG: s * PSEG + n] = point_fea[rows]
            pslot[s * PSEG: s * PSEG + n, 0] = (
                sorted_vox[a:b] - (c * Ms + s * 512)).astype(np.float32)
        rcp = np.zeros((MsP, 1), np.float32)
        rcp[:Ms, 0] = recip_full[vs]
        nb28 = np.full((MsP, 27), ZR, np.int32)
        nb28[:Ms] = nb_remap[vs]
        didx = np.full((NpP, KD), ZR, np.int32)
        didx[:Np] = _remap(idx_dev[ps])
        wd = np.zeros((NpP, KD), np.float32)
        wd[:Np] = w_dev[ps]
        per.append(dict(pfseg=pfseg, pslot=pslot, rcp=rcp, nb28=nb28,
                        didx=didx, wd=wd))

    W1s = _stack_w(np.asarray(W_s1), 1)
    W2s = _stack_w(np.asarray(W_s2), 7)
    Wr1s = _stack_w(np.asarray(W_r1), 7)
    Wr2s = _stack_w(np.asarray(W_r2), 7)
    Wc_pad = np.zeros((C0, C0), np.float32)
    Wc_pad[:, :NCLS] = np.asarray(W_c)
    bc_pad = np.zeros((1, C0), np.float32)
    bc_pad[0, :NCLS] = np.asarray(b_c)
    gbm = np.stack([np.asarray(g_s1), np.asarray(b_s1),
                    np.asarray(g_s2), np.asarray(b_s2),
                    np.asarray(g_r1), np.asarray(b_r1),
                    np.asarray(g_r2), np.asarray(b_r2)],
                   axis=1).astype(np.float32)          # [32, 8]
    iot5 = np.arange(512, dtype=np.float32)[None, :]
    pm = np.zeros((128, SUP), np.float32)
    base = (NSUP_V - 1) * 512
    for t in range(SUP):
        for p in range(128):
            pm[p, t] = 1.0 if base + t * 128 + p < Ms else 0.0

    R = _get_runner()
    res = R([dict(pfseg=per[c]["pfseg"], pslot=per[c]["pslot"],
                  rcp=per[c]["rcp"], nbrs=per[c]["nb28"],
                  didx=per[c]["didx"], wdev=per[c]["wd"],
                  w1=W1s, w2=W2s, wr1=Wr1s, wr2=Wr2s,
                  gb=gbm, wc=Wc_pad, bc=bc_pad, iot5=iot5, pmask=pm)
             for c in range(NC)])
    out = np.concatenate([res[c]["out"][:Np] for c in range(NC)], 0)
    return np.ascontiguousarray(out)

